# revision 16
# baseline (speedup 1.0000x reference)
"""Multi-head attention (B=2, T=2048, d_model=1024, H=16) on 8 TRN2 NeuronCores.

Sharding: core c owns batch b=c//4 and heads 4*(c%4)..4*(c%4)+3, split into
two head-pairs p in {0,1}.  Within a pair, head h in {0,1} lives on SBUF
partitions 64h..64h+63 (feature-major q/k), enabling 2-head row-group
concurrent score matmuls and split-K concurrent PV matmuls on the PE array.

Pipeline: scores (PE, PSUM rotation) -> exp (split between ScalarE `Exp`
and a custom 2-pass VectorE exp2 built from float-only ops + a bf16
bitcast view) -> PV (PE, ones-column yields softmax denominators) ->
normalize (recip_approx + gpsimd broadcast/mult) -> AllToAll (token
resharding, bf16) -> output projection.  QKV / output projections are
software-pipelined into the score/exp rotation as filler units stealing
PSUM rotation buffers, keeping the PE busy under the exp stream.

All matmuls bf16 (fp32 PSUM accumulation).  Wq is pre-scaled on the host by
log2e/8 so the softmax is exp2-based on both engines.
"""

import math
from collections import deque

import numpy as np

import concourse.bass as bass
import concourse.mybir as mybir
import concourse.tile as tile
from concourse import bacc, library_config

# ---------------------------------------------------------------- geometry
B, T, D = 2, 2048, 1024
H, DK = 16, 64
NCORES = 8
GROUPS = NCORES // B         # 4 head-groups per batch
HPC = H // GROUPS            # 4 heads per core (2 pairs)
FPC = HPC * DK               # 256
TOUT = T // NCORES           # 256 output tokens per core per batch
QB = 512                     # query block (scores free dim)
NQB = T // QB                # 4
NKC = T // 128               # 16 key chunks

F32 = mybir.dt.float32
BF16 = mybir.dt.bfloat16
I32 = mybir.dt.int32

LN2 = 0.6931471805599453
LOG2E = 1.4426950408889634

# ------------------------------------------------------------- config
G = 3                        # slices per exp group (PSUM banks per rot tile)
SPLIT_PV = False             # split-K concurrent PV (needs 2*2 pv banks)
EXP_PATTERN = "A"            # per-group engine: A=ScalarE, D=VectorE
FILLER_CADENCE = 2           # emit 1 proj unit per this many groups

# exp2 custom-op constants
MAGIC = float(3 * 2 ** 22)                  # round-to-int magic
E2A_C1 = 128.0
E2A_C2 = float(3 * 2 ** 22 + 127 * 128)     # exponent-field assembler
E2B_A = 0.70294179                          # 2^f ~ 1 + f*(A + B*f), |f|<=.5
E2B_B = 0.23986403

# ---------------------------------------------------- custom DVE exp2 ops
_EXP2_OPS = {}


def _register_exp2_ops():
    if _EXP2_OPS:
        return _EXP2_OPS
    import concourse.dve_ops as dve_ops
    from concourse.dve_spec import Spec, Src0, Src1, C0, C1, C2, One, lower
    from concourse.dve_uop import DveOpSpec

    def _refA(in0, in1, c0, c1, c2):
        z = (in0.astype(np.float32) + np.float32(c0)).astype(np.float32)
        rb = (z - np.float32(c0)).astype(np.float32)
        return (rb * np.float32(c1) + np.float32(c2)).astype(np.float32)

    def _refB(in0, in1, c0, c1, c2):
        x = in0.astype(np.float32)
        z = (x + np.float32(c0)).astype(np.float32)
        rb = (z - np.float32(c0)).astype(np.float32)
        f = (x - rb).astype(np.float32)
        p = 1.0 + f * (np.float32(c2) + np.float32(c1) * f)
        return (p * in1.astype(np.float32)).astype(np.float32)

    _zA = Src0 + C0
    specA = Spec(body=(_zA - C0) * C1 + C2, reference=_refA)
    _zB = Src0 + C0
    _fB = Src0 - (_zB - C0)
    specB = Spec(body=(((_fB * C1) + C2) * _fB + One) * Src1, reference=_refB)

    def _mk(name, spec):
        for op in dve_ops.OPS:
            if op.name == name:
                return op
        row = dve_ops._CUSTOM_DVE_ROW_BASE + len(dve_ops.OPS)
        assert row < 0x20
        dve_ops._SUB_OPCODE_FOR_NAME[name] = row
        shas = {}
        from concourse.dve_spec import _has_src1
        for ver in ("v3", "v4"):
            s = DveOpSpec(name=name, opcode=row, uops=lower(spec, ver=ver),
                          rd1_en=_has_src1(spec))
            shas[ver] = s.sha(ver)
        op = dve_ops.DveOp(name, spec, False, shas)
        dve_ops.OPS.append(op)
        dve_ops.CUSTOM_DVE_SPECS[name] = spec
        return op

    _EXP2_OPS["A"] = _mk("ANT_EXP2_PASS_A", specA)
    _EXP2_OPS["B"] = _mk("ANT_EXP2_PASS_B", specB)
    return _EXP2_OPS


# ------------------------------------------------------------------ build
def build_nc(reps: int = 1, add_bias: bool = False, debug: bool = False) -> bass.Bass:
    if "D" in EXP_PATTERN:
        _register_exp2_ops()
    nc = bacc.Bacc("TRN2", target_bir_lowering=False, num_devices=NCORES)

    xT = nc.dram_tensor("xT", [D, T], BF16, kind="ExternalInput")
    wq = nc.dram_tensor("wq", [D, FPC], BF16, kind="ExternalInput")
    wk = nc.dram_tensor("wk", [D, FPC], BF16, kind="ExternalInput")
    wv = nc.dram_tensor("wv", [D, FPC], BF16, kind="ExternalInput")
    wout = nc.dram_tensor("wout", [D, D], BF16, kind="ExternalInput")
    out = nc.dram_tensor("out", [B, TOUT, D], F32, kind="ExternalOutput")
    if add_bias:
        bq = nc.dram_tensor("bq", [1, FPC], BF16, kind="ExternalInput")
        bk = nc.dram_tensor("bk", [1, FPC], BF16, kind="ExternalInput")
        bv = nc.dram_tensor("bv", [1, FPC], BF16, kind="ExternalInput")
        bout = nc.dram_tensor("bout", [1, D], BF16, kind="ExternalInput")
    else:
        bq = bk = bv = bout = None

    with tile.TileContext(nc, num_cores=NCORES) as tc:
        with (
            tc.tile_pool(name="persist", bufs=1) as pers,
            tc.tile_pool(name="dram", bufs=1, space="DRAM") as dram,
        ):
            nc.gpsimd.load_library(library_config.attn)

            st = {}
            st["xT_sb"] = pers.tile([128, 8, T], BF16, name="xT_sb")
            st["wq_sb"] = pers.tile([128, 8, FPC], BF16, name="wq_sb")
            st["wk_sb"] = pers.tile([128, 8, FPC], BF16, name="wk_sb")
            st["wv_sb"] = pers.tile([128, 8, FPC], BF16, name="wv_sb")
            st["wout_sb"] = pers.tile([128, 8, D], BF16, name="wout_sb")
            st["qT"] = pers.tile([128, 2, T], BF16, name="qT")
            st["kT"] = pers.tile([128, 2, T], BF16, name="kT")
            st["vsb"] = pers.tile([128, NKC, HPC, 2 * DK], BF16, name="vsb")
            st["ctx"] = pers.tile([128, 2, T], BF16, name="ctx")
            st["acc"] = pers.tile([128, 8, 512], F32, name="acc")
            st["ones"] = pers.tile([1, QB], BF16, name="ones")
            if add_bias:
                st["bq_sb"] = pers.tile([1, FPC], BF16, name="bq_sb")
                st["bk_sb"] = pers.tile([1, FPC], BF16, name="bk_sb")
                st["bv_sb"] = pers.tile([1, FPC], BF16, name="bv_sb")
                st["bout_sb"] = pers.tile([1, D], BF16, name="bout_sb")

            nc.vector.memset(st["ones"][:], 1.0)
            nc.vector.memset(st["vsb"][:, :, :, DK:2 * DK], 1.0)

            # weights: once (persist across reps)
            for ko in range(8):
                nc.sync.dma_start(st["wk_sb"][:, ko, :],
                                  wk[ko * 128:(ko + 1) * 128, :])
                nc.sync.dma_start(st["wq_sb"][:, ko, :],
                                  wq[ko * 128:(ko + 1) * 128, :])
            for ko in range(8):
                nc.sync.dma_start(st["wv_sb"][:, ko, :],
                                  wv[ko * 128:(ko + 1) * 128, :])
            if add_bias:
                nc.sync.dma_start(st["bq_sb"][:], bq[:, :])
                nc.sync.dma_start(st["bk_sb"][:], bk[:, :])
                nc.sync.dma_start(st["bv_sb"][:], bv[:, :])
                nc.sync.dma_start(st["bout_sb"][:], bout[:, :])
            for ko in range(8):
                nc.sync.dma_start(st["wout_sb"][:, ko, :],
                                  wout[ko * 128:(ko + 1) * 128, :])

            if debug:
                dbg = {
                    "dbg_a2a": nc.dram_tensor("dbg_a2a", [2, NCORES * 128, TOUT],
                                              BF16, kind="ExternalOutput"),
                    "dbg_qT": nc.dram_tensor("dbg_qT", [128, 2, T], BF16,
                                             kind="ExternalOutput"),
                    "dbg_kT": nc.dram_tensor("dbg_kT", [128, 2, T], BF16,
                                             kind="ExternalOutput"),
                    "dbg_vsb": nc.dram_tensor("dbg_vsb",
                                              [128, NKC, HPC, 2 * DK], BF16,
                                              kind="ExternalOutput"),
                    "dbg_ctx": nc.dram_tensor("dbg_ctx", [128, 2, T], BF16,
                                              kind="ExternalOutput"),
                }
                st["dbg"] = dbg
            for _rep in range(reps):
                _emit_body(nc, tc, dram, st, xT, out, add_bias)
            if debug:
                nc.sync.dma_start(st["dbg"]["dbg_qT"][:, :, :], st["qT"][:])
                nc.sync.dma_start(st["dbg"]["dbg_kT"][:, :, :], st["kT"][:])
                nc.sync.dma_start(st["dbg"]["dbg_vsb"][:, :, :, :], st["vsb"][:])
                nc.sync.dma_start(st["dbg"]["dbg_ctx"][:, :, :], st["ctx"][:])

    nc.finalize()
    return nc


def _emit_body(nc, tc, dram, st, xT, out, add_bias):
    xT_sb, qT, kT, vsb, ctx, acc = (st["xT_sb"], st["qT"], st["kT"],
                                    st["vsb"], st["ctx"], st["acc"])
    wq_sb, wk_sb, wv_sb, wout_sb = (st["wq_sb"], st["wk_sb"], st["wv_sb"],
                                    st["wout_sb"])
    ones = st["ones"]
    Exp = mybir.ActivationFunctionType.Exp
    MULT = mybir.AluOpType.mult
    ADD = mybir.AluOpType.add

    a2a_in = [dram.tile([NCORES * 128, TOUT], BF16, name=f"a2a_in{p}")
              for p in range(2)]
    a2a_out = [dram.tile([NCORES * 128, TOUT], BF16, name=f"a2a_out{p}")
               for p in range(2)]

    with (
        tc.tile_pool(name="rotp", bufs=2, space="PSUM") as rotp,
        tc.tile_pool(name="pvp", bufs=1, space="PSUM") as pvp,
        tc.tile_pool(name="prp", bufs=3) as prp,
        tc.tile_pool(name="ep", bufs=2) as ep,
        tc.tile_pool(name="stp", bufs=2) as stp,
        tc.tile_pool(name="rcpp", bufs=2) as rcpp,
        tc.tile_pool(name="bcp", bufs=2) as bcp,
        tc.tile_pool(name="ctinp", bufs=16) as ctinp,
        tc.tile_pool(name="ostgp", bufs=3) as ostgp,
    ):
        # ---------------- input DMA (qb-major for early compute start)
        for qb in range(NQB):
            for ko in range(8):
                nc.sync.dma_start(
                    xT_sb[:, ko, qb * QB:(qb + 1) * QB],
                    xT[ko * 128:(ko + 1) * 128, qb * QB:(qb + 1) * QB])

        exp_ops = _EXP2_OPS if "D" in EXP_PATTERN else None

        def rot_tile():
            return rotp.tile([128, G, QB], F32, name="rot", tag="rot")

        def proj_mms(sub, wmat, p, qb, bias_t):
            """q/k proj unit: sub [128, 512] <- w_p.T @ xT (8 ko chunks)."""
            if add_bias:
                nc.tensor.matmul(sub, bias_t[:, p * 128:(p + 1) * 128],
                                 ones[:, :QB], start=True, stop=False)
            for ko in range(8):
                nc.tensor.matmul(
                    sub,
                    wmat[:, ko, p * 128:(p + 1) * 128],
                    xT_sb[:, ko, qb * QB:(qb + 1) * QB],
                    start=(ko == 0 and not add_bias), stop=(ko == 7))

        def unit_qk(mat, p, qb):
            dst = qT if mat == "q" else kT
            wmat = wq_sb if mat == "q" else wk_sb
            bias_t = st.get("bq_sb" if mat == "q" else "bk_sb")
            rt = rot_tile()
            sub = rt[:, 0, :]
            proj_mms(sub, wmat, p, qb, bias_t)
            nc.vector.tensor_copy(dst[:, p, qb * QB:(qb + 1) * QB], sub)

        def unit_qk2(mat, p, qb2):
            """q/k proj for qb pair (2*qb2, 2*qb2+1), ko-outer so the
            stationary w chunk is reused across the two 512-col MMs."""
            dst = qT if mat == "q" else kT
            wmat = wq_sb if mat == "q" else wk_sb
            bias_t = st.get("bq_sb" if mat == "q" else "bk_sb")
            rt = rot_tile()
            if add_bias:
                for j in range(2):
                    nc.tensor.matmul(rt[:, j, :],
                                     bias_t[:, p * 128:(p + 1) * 128],
                                     ones[:, :QB], start=True, stop=False)
            for ko in range(8):
                for j in range(2):
                    nc.tensor.matmul(
                        rt[:, j, :],
                        wmat[:, ko, p * 128:(p + 1) * 128],
                        xT_sb[:, ko, (2 * qb2 + j) * QB:(2 * qb2 + j + 1) * QB],
                        start=(ko == 0 and not add_bias), stop=(ko == 7))
            for j in range(2):
                qb = 2 * qb2 + j
                nc.vector.tensor_copy(dst[:, p, qb * QB:(qb + 1) * QB],
                                      rt[:, j, :])

        def unit_v(p, quarter):
            rt = rot_tile()
            for tq in range(4):
                t = quarter * 4 + tq
                sub = rt[:, 0, tq * 128:(tq + 1) * 128]
                if add_bias:
                    nc.tensor.matmul(sub, ones[:, :128],
                                     st["bv_sb"][:, p * 128:(p + 1) * 128],
                                     start=True, stop=False)
                for ko in range(8):
                    nc.tensor.matmul(
                        sub,
                        xT_sb[:, ko, t * 128:(t + 1) * 128],
                        wv_sb[:, ko, p * 128:(p + 1) * 128],
                        start=(ko == 0 and not add_bias), stop=(ko == 7))
            nc.vector.tensor_copy(
                vsb[:, quarter * 4:quarter * 4 + 4, 2 * p:2 * p + 2, 0:DK],
                rt[:, 0, :].rearrange("p (t h d) -> p t h d", t=4, h=2, d=DK))

        ctin = {}

        def unit_op(phase, b, t2, nf):
            """out-proj unit: pair-0 partial (-> acc) or final (+acc -> out)."""
            rt = rot_tile()
            sub = rt[:, 0, :]
            p = 0 if phase == "partial" else 1
            fos = [2 * g_ + p for g_ in range(4)]
            first = True
            if phase == "final" and add_bias:
                nc.tensor.matmul(sub, ones[:, :128],
                                 st["bout_sb"][:, nf * 512:(nf + 1) * 512],
                                 start=True, stop=False)
                first = False
            for i, fo in enumerate(fos):
                nc.tensor.matmul(
                    sub,
                    ctin[(b, fo)][:, t2 * 128:(t2 + 1) * 128],
                    wout_sb[:, fo, nf * 512:(nf + 1) * 512],
                    start=(i == 0 and first), stop=(i == 3))
            idx = b * 4 + t2 * 2 + nf
            if phase == "partial":
                nc.vector.tensor_copy(acc[:, idx, :], sub)
            else:
                og = ostgp.tile([128, 512], F32, name="ostg", tag="ostg")
                nc.vector.tensor_tensor(og[:], sub, acc[:, idx, :], ADD)
                nc.sync.dma_start(
                    out[b, t2 * 128:(t2 + 1) * 128, nf * 512:(nf + 1) * 512],
                    og[:])

        # ---------------- filler queue (writers must precede their readers
        # in emission order; overlap comes from the engine-level schedule)
        filler = deque()
        filler.append(("qk2", "q", 0, 1))
        for qb2 in range(2):
            filler.append(("qk2", "k", 1, qb2))
        for qb2 in range(2):
            filler.append(("qk2", "q", 1, qb2))
        for quarter in range(4):
            filler.append(("v", 1, quarter))

        def run_unit(u):
            kind = u[0]
            if kind == "qk":
                unit_qk(u[1], u[2], u[3])
            elif kind == "qk2":
                unit_qk2(u[1], u[2], u[3])
            elif kind == "v":
                unit_v(u[1], u[2])
            elif kind == "op":
                unit_op(u[1], u[2], u[3], u[4])
            elif kind == "ctin":
                _, p = u
                for b in range(B):
                    for g_ in range(4):
                        fo = 2 * g_ + p
                        t_ = ctinp.tile([128, TOUT], BF16,
                                        name=f"ctin{b}_{fo}", tag="ctin")
                        nc.sync.dma_start(
                            t_[:],
                            a2a_out[p][(b * 4 + g_) * 128:(b * 4 + g_ + 1) * 128, :])
                        ctin[(b, fo)] = t_

        # ---------------- prologue: K(p0) all, Q(p0, qb0+qb1)
        unit_qk2("k", 0, 0)
        unit_qk2("k", 0, 1)
        unit_qk2("q", 0, 0)

        # ---------------- phase 2
        expctr = [0]

        def emit_exp(n, rt, pr):
            eng = EXP_PATTERN[expctr[0] % len(EXP_PATTERN)]
            expctr[0] += 1
            if eng == "A" or exp_ops is None:
                nc.scalar.activation(pr[:, 0:n, :], rt[:, 0:n, :], Exp,
                                     scale=LN2)
            else:
                et = ep.tile([128, G * QB], F32, name="e2", tag="e2")
                nc.vector._custom_dve(
                    exp_ops["A"], out=et[:, 0:n * QB], in0=rt[:, 0:n, :],
                    s0=MAGIC, s1=E2A_C1, imm2=E2A_C2)
                ebf = et[:].bitcast(BF16)
                nc.vector._custom_dve(
                    exp_ops["B"], out=pr[:, 0:n, :], in0=rt[:, 0:n, :],
                    in1=ebf[:, 0:2 * n * QB:2],
                    s0=MAGIC, s1=E2B_B, imm2=E2B_A)

        def emit_pv(p, grp, pr, pvt):
            for j, (kc, h) in enumerate(grp):
                if SPLIT_PV:
                    for half in range(2):
                        nc.tensor.matmul(
                            pvt[h][:, half, :],
                            vsb[64 * half:64 * half + 64, kc, 2 * p + h, :],
                            pr[64 * half:64 * half + 64, j, :],
                            start=(kc == 0), stop=(kc == NKC - 1))
                else:
                    nc.tensor.matmul(
                        pvt[h][:, 0, :],
                        vsb[:, kc, 2 * p + h, :],
                        pr[:, j, :],
                        start=(kc == 0), stop=(kc == NKC - 1))

        nsp = 2 if SPLIT_PV else 1
        fill_tick = [0]

        def maybe_fill():
            fill_tick[0] += 1
            if fill_tick[0] % FILLER_CADENCE == 0 and filler:
                run_unit(filler.popleft())

        for p in range(2):
            if p == 1:
                # all pair-1 proj writers must precede pair-1 readers
                while filler and filler[0][0] in ("qk", "qk2", "v"):
                    run_unit(filler.popleft())
            for qb in range(NQB):
                slices = [(kc, h) for kc in range(NKC) for h in range(2)]
                groups = [slices[i:i + G] for i in range(0, len(slices), G)]
                hist = deque()  # (grp, rot, pr)
                pvt = {h: pvp.tile([128, nsp, QB], F32,
                                   name=f"pv{h}", tag=f"pv{h}")
                       for h in range(2)}
                for gi, grp in enumerate(groups):
                    if p == 0 and qb == 0 and gi in (0, 2, 4, 6):
                        # vsb quarter q written before the PV that reads it
                        unit_v(0, gi // 2)
                    rt = rot_tile()
                    for j, (kc, h) in enumerate(grp):
                        nc.tensor.matmul(
                            rt[:, j, :],
                            kT[64 * h:64 * h + 64, p, kc * 128:(kc + 1) * 128],
                            qT[64 * h:64 * h + 64, p, qb * QB:(qb + 1) * QB],
                            start=True, stop=True,
                            tile_position=(64 * h, 0))
                    pr = prp.tile([128, G, QB], BF16, name="pr", tag="pr")
                    hist.append((grp, rt, pr))
                    if len(hist) >= 2:
                        g2, r2, p2 = hist[-2]
                        emit_exp(len(g2), r2, p2)
                    if len(hist) >= 3:
                        g3, _, p3 = hist.popleft()
                        emit_pv(p, g3, p3, pvt)
                    maybe_fill()
                # drain
                g2, r2, p2 = hist[-1]
                emit_exp(len(g2), r2, p2)
                while hist:
                    g3, _, p3 = hist.popleft()
                    emit_pv(p, g3, p3, pvt)
                # normalize
                for h in range(2):
                    if SPLIT_PV:
                        s_ = stp.tile([128, QB], F32, name="st", tag="st")
                        nc.vector.tensor_tensor(s_[:], pvt[h][:, 0, :],
                                                pvt[h][:, 1, :], ADD)
                        base = s_[:]
                    else:
                        base = pvt[h][:, 0, :]
                    # rows 64-127 hold the denominator (64 ones-columns in V)
                    den = rcpp.tile([DK, QB], F32, name="den", tag="den")
                    nc.vector.tensor_copy(den[:], base[DK:2 * DK, :])
                    rcp_t = bcp.tile([DK, QB], F32, name="rcp2", tag="rcp2")
                    nc.vector.reciprocal_approx_fast(rcp_t[:], den[:])
                    nc.vector.tensor_tensor(
                        ctx[64 * h:64 * h + 64, p, qb * QB:(qb + 1) * QB],
                        base[0:DK, :], rcp_t[:], MULT)
            # pair done: exchange token-wise
            for j in range(NCORES):
                nc.sync.dma_start(a2a_in[p][j * 128:(j + 1) * 128, :],
                                  ctx[:, p, j * TOUT:(j + 1) * TOUT])
            nc.gpsimd.collective_compute(
                "AllToAll", mybir.AluOpType.bypass,
                replica_groups=[list(range(NCORES))],
                ins=[a2a_in[p][:].opt()],
                outs=[a2a_out[p][:].opt()])
            if p == 0:
                filler.append(("ctin", 0))
                for b in range(B):
                    for t2 in range(2):
                        for nf in range(2):
                            filler.append(("op", "partial", b, t2, nf))

        # ---------------- tail: drain filler, then final out-proj
        while filler:
            run_unit(filler.popleft())
        run_unit(("ctin", 1))
        if "dbg" in st:
            for p_ in range(2):
                nc.sync.dma_start(st["dbg"]["dbg_a2a"][p_, :, :],
                                  a2a_out[p_][:, :])
        for b in range(B):
            for t2 in range(2):
                for nf in range(2):
                    unit_op("final", b, t2, nf)


# ------------------------------------------------------------------ host
def make_in_maps(x, Wqkv, bqkv, Wout, bout):
    import ml_dtypes
    bf16 = ml_dtypes.bfloat16
    x = np.asarray(x, dtype=np.float32)
    Wqkv = np.asarray(Wqkv, dtype=np.float32)
    bqkv = np.asarray(bqkv, dtype=np.float32)
    Wout = np.asarray(Wout, dtype=np.float32)
    bout = np.asarray(bout, dtype=np.float32)
    add_bias = bool(np.any(bqkv) or np.any(bout))

    qscale = LOG2E / math.sqrt(DK)
    xT_all = np.ascontiguousarray(np.transpose(x, (0, 2, 1))).astype(bf16)
    wout_b = np.ascontiguousarray(Wout).astype(bf16)
    in_maps = []
    for c in range(NCORES):
        b = c // GROUPS
        h0 = HPC * (c % GROUPS)
        fsl = slice(h0 * DK, h0 * DK + FPC)
        m = {
            "xT": xT_all[b],
            "wq": np.ascontiguousarray(Wqkv[:, 0 * D:1 * D][:, fsl]
                                       * qscale).astype(bf16),
            "wk": np.ascontiguousarray(Wqkv[:, 1 * D:2 * D][:, fsl]).astype(bf16),
            "wv": np.ascontiguousarray(Wqkv[:, 2 * D:3 * D][:, fsl]).astype(bf16),
            "wout": wout_b,
        }
        if add_bias:
            m["bq"] = (np.ascontiguousarray(bqkv[0 * D:1 * D][fsl])[None, :]
                       * qscale).astype(bf16)
            m["bk"] = np.ascontiguousarray(
                bqkv[1 * D:2 * D][fsl])[None, :].astype(bf16)
            m["bv"] = np.ascontiguousarray(
                bqkv[2 * D:3 * D][fsl])[None, :].astype(bf16)
            m["bout"] = bout[None, :].astype(bf16)
        in_maps.append(m)
    return in_maps


_CACHE = {}


def _get_runner(reps: int = 1, add_bias: bool = False):
    key = ("runner", reps, add_bias)
    if key in _CACHE:
        return _CACHE[key]

    import jax
    from jax.experimental.shard_map import shard_map
    from jax.sharding import Mesh, PartitionSpec
    from concourse import bass2jax
    from concourse import mybir as _mybir

    nc = build_nc(reps=reps, add_bias=add_bias)
    bass2jax.install_neuronx_cc_hook()

    partition_name = nc.partition_id_tensor.name if nc.partition_id_tensor else None
    in_names, out_names, out_avals = [], [], []
    for alloc in nc.m.functions[0].allocations:
        if not isinstance(alloc, _mybir.MemoryLocationSet):
            continue
        name = alloc.memorylocations[0].name
        if alloc.kind == "ExternalInput":
            if name != partition_name:
                in_names.append(name)
        elif alloc.kind == "ExternalOutput":
            out_names.append(name)
            out_avals.append(
                jax.core.ShapedArray(
                    tuple(alloc.tensor_shape), _mybir.dt.np(alloc.dtype)))
    n_params = len(in_names)
    all_in_names = list(in_names) + list(out_names)
    if partition_name is not None:
        all_in_names.append(partition_name)

    def _body(*args):
        operands = list(args)
        if partition_name is not None:
            operands.append(bass2jax.partition_id_tensor())
        outs = bass2jax._bass_exec_p.bind(
            *operands,
            out_avals=tuple(out_avals),
            in_names=tuple(all_in_names),
            out_names=tuple(out_names),
            lowering_input_output_aliases=(),
            sim_require_finite=False,
            sim_require_nnan=False,
            nc=nc,
        )
        return tuple(outs)

    devices = jax.devices()[:NCORES]
    mesh = Mesh(np.asarray(devices), ("core",))
    n_outs = len(out_names)
    fn = jax.jit(
        shard_map(
            _body,
            mesh=mesh,
            in_specs=(PartitionSpec("core"),) * (n_params + n_outs),
            out_specs=(PartitionSpec("core"),) * n_outs,
            check_rep=False,
        ),
        keep_unused=True,
    )

    def run(in_maps):
        concat_in = [
            np.concatenate([np.asarray(in_maps[c][nm]) for c in range(NCORES)],
                           axis=0)
            for nm in in_names
        ]
        zeros = [
            np.zeros((NCORES * av.shape[0], *av.shape[1:]), av.dtype)
            for av in out_avals
        ]
        out_arrs = fn(*concat_in, *zeros)
        return [
            {nm: np.asarray(out_arrs[i]).reshape(NCORES, *out_avals[i].shape)[c]
             for i, nm in enumerate(out_names)}
            for c in range(NCORES)
        ]

    runner = {"run": run, "fn": fn, "in_names": in_names,
              "out_avals": out_avals, "out_names": out_names,
              "n_params": n_params, "mesh": mesh}
    _CACHE[key] = runner
    return runner


def kernel(x, Wqkv, bqkv, Wout, bout) -> np.ndarray:
    add_bias = bool(np.any(np.asarray(bqkv)) or np.any(np.asarray(bout)))
    runner = _get_runner(add_bias=add_bias)
    in_maps = make_in_maps(x, Wqkv, bqkv, Wout, bout)
    results = runner["run"](in_maps)
    full = np.empty((B, T, D), dtype=np.float32)
    for c in range(NCORES):
        full[:, c * TOUT:(c + 1) * TOUT, :] = results[c]["out"]
    return full


# revision 18
# speedup vs baseline: 1.1162x; 1.1162x over previous
"""Multi-head attention (B=2, T=2048, d_model=1024, H=16) on 8 TRN2 NeuronCores.

Sharding: core c owns batch b=c//4 and heads 4*(c%4)..4*(c%4)+3, split into
two head-pairs p in {0,1}.  Within a pair, head h in {0,1} lives on SBUF
partitions 64h..64h+63 (feature-major q/k), enabling 2-head row-group
concurrent score matmuls and split-K concurrent PV matmuls on the PE array.

Pipeline: scores (PE, 3-bank double-buffered PSUM rotation) -> exp2 on
ScalarE (`Exp` with scale=ln2; Wq is host-prescaled by log2e/8) -> PV
(PE; V carries 64 ones-columns so partitions 64-127 of the PV
accumulator replicate the softmax denominator, lane-aligned for the
cheap `reciprocal_approx_fast` normalize - no gpsimd broadcast, no
iterative divide) -> AllToAll (token resharding, bf16 payload) ->
output projection (pair-0 partials accumulated into SBUF under pair-1
attention; pair-1 finals in the tail).  QKV / output projections are
software-pipelined into the score/exp rotation as filler units stealing
PSUM rotation buffers, keeping the PE busy under the exp stream; all
proj writers are emitted before their readers (program-order semantics).

An optional custom 2-pass VectorE exp2 (float-only ops + a bf16
bitcast view of the exponent field) is registered when EXP_PATTERN
contains "D", to offload part of the exp stream to the DVE; the final
configuration is ScalarE-only since the PE, not ScalarE, is critical.

All matmuls bf16 (fp32 PSUM accumulation).  Wq is pre-scaled on the host by
log2e/8 so the softmax is exp2-based on both engines.
"""

import math
from collections import deque

import numpy as np

import concourse.bass as bass
import concourse.mybir as mybir
import concourse.tile as tile
from concourse import bacc, library_config

# ---------------------------------------------------------------- geometry
B, T, D = 2, 2048, 1024
H, DK = 16, 64
NCORES = 8
GROUPS = NCORES // B         # 4 head-groups per batch
HPC = H // GROUPS            # 4 heads per core (2 pairs)
FPC = HPC * DK               # 256
TOUT = T // NCORES           # 256 output tokens per core per batch
QB = 512                     # query block (scores free dim)
NQB = T // QB                # 4
NKC = T // 128               # 16 key chunks

F32 = mybir.dt.float32
BF16 = mybir.dt.bfloat16
I32 = mybir.dt.int32

LN2 = 0.6931471805599453
LOG2E = 1.4426950408889634

# ------------------------------------------------------------- config
G = 3                        # slices per exp group (PSUM banks per rot tile)
SPLIT_PV = False             # split-K concurrent PV (needs 2*2 pv banks)
EXP_PATTERN = "A"            # per-group engine: A=ScalarE, D=VectorE
FILLER_CADENCE = 2           # emit 1 proj unit per this many groups

# exp2 custom-op constants
MAGIC = float(3 * 2 ** 22)                  # round-to-int magic
E2A_C1 = 128.0
E2A_C2 = float(3 * 2 ** 22 + 127 * 128)     # exponent-field assembler
E2B_A = 0.70294179                          # 2^f ~ 1 + f*(A + B*f), |f|<=.5
E2B_B = 0.23986403

# ---------------------------------------------------- custom DVE exp2 ops
_EXP2_OPS = {}


def _register_exp2_ops():
    if _EXP2_OPS:
        return _EXP2_OPS
    import concourse.dve_ops as dve_ops
    from concourse.dve_spec import Spec, Src0, Src1, C0, C1, C2, One, lower
    from concourse.dve_uop import DveOpSpec

    def _refA(in0, in1, c0, c1, c2):
        z = (in0.astype(np.float32) + np.float32(c0)).astype(np.float32)
        rb = (z - np.float32(c0)).astype(np.float32)
        return (rb * np.float32(c1) + np.float32(c2)).astype(np.float32)

    def _refB(in0, in1, c0, c1, c2):
        x = in0.astype(np.float32)
        z = (x + np.float32(c0)).astype(np.float32)
        rb = (z - np.float32(c0)).astype(np.float32)
        f = (x - rb).astype(np.float32)
        p = 1.0 + f * (np.float32(c2) + np.float32(c1) * f)
        return (p * in1.astype(np.float32)).astype(np.float32)

    _zA = Src0 + C0
    specA = Spec(body=(_zA - C0) * C1 + C2, reference=_refA)
    _zB = Src0 + C0
    _fB = Src0 - (_zB - C0)
    specB = Spec(body=(((_fB * C1) + C2) * _fB + One) * Src1, reference=_refB)

    def _mk(name, spec):
        for op in dve_ops.OPS:
            if op.name == name:
                return op
        row = dve_ops._CUSTOM_DVE_ROW_BASE + len(dve_ops.OPS)
        assert row < 0x20
        dve_ops._SUB_OPCODE_FOR_NAME[name] = row
        shas = {}
        from concourse.dve_spec import _has_src1
        for ver in ("v3", "v4"):
            s = DveOpSpec(name=name, opcode=row, uops=lower(spec, ver=ver),
                          rd1_en=_has_src1(spec))
            shas[ver] = s.sha(ver)
        op = dve_ops.DveOp(name, spec, False, shas)
        dve_ops.OPS.append(op)
        dve_ops.CUSTOM_DVE_SPECS[name] = spec
        return op

    _EXP2_OPS["A"] = _mk("ANT_EXP2_PASS_A", specA)
    _EXP2_OPS["B"] = _mk("ANT_EXP2_PASS_B", specB)
    return _EXP2_OPS


# ------------------------------------------------------------------ build
def build_nc(reps: int = 1, add_bias: bool = False, debug: bool = False) -> bass.Bass:
    if "D" in EXP_PATTERN:
        _register_exp2_ops()
    nc = bacc.Bacc("TRN2", target_bir_lowering=False, num_devices=NCORES)

    xT = nc.dram_tensor("xT", [D, T], BF16, kind="ExternalInput")
    wq = nc.dram_tensor("wq", [D, FPC], BF16, kind="ExternalInput")
    wk = nc.dram_tensor("wk", [D, FPC], BF16, kind="ExternalInput")
    wv = nc.dram_tensor("wv", [D, FPC], BF16, kind="ExternalInput")
    wout = nc.dram_tensor("wout", [D, D], BF16, kind="ExternalInput")
    out = nc.dram_tensor("out", [B, TOUT, D], F32, kind="ExternalOutput")
    if add_bias:
        bq = nc.dram_tensor("bq", [1, FPC], BF16, kind="ExternalInput")
        bk = nc.dram_tensor("bk", [1, FPC], BF16, kind="ExternalInput")
        bv = nc.dram_tensor("bv", [1, FPC], BF16, kind="ExternalInput")
        bout = nc.dram_tensor("bout", [1, D], BF16, kind="ExternalInput")
    else:
        bq = bk = bv = bout = None

    with tile.TileContext(nc, num_cores=NCORES) as tc:
        with (
            tc.tile_pool(name="persist", bufs=1) as pers,
            tc.tile_pool(name="dram", bufs=1, space="DRAM") as dram,
        ):
            nc.gpsimd.load_library(library_config.attn)

            st = {}
            st["xT_sb"] = pers.tile([128, 8, T], BF16, name="xT_sb")
            st["wq_sb"] = pers.tile([128, 8, FPC], BF16, name="wq_sb")
            st["wk_sb"] = pers.tile([128, 8, FPC], BF16, name="wk_sb")
            st["wv_sb"] = pers.tile([128, 8, FPC], BF16, name="wv_sb")
            st["wout_sb"] = pers.tile([128, 8, D], BF16, name="wout_sb")
            st["qT"] = pers.tile([128, 2, T], BF16, name="qT")
            st["kT"] = pers.tile([128, 2, T], BF16, name="kT")
            st["vsb"] = pers.tile([128, NKC, HPC, 2 * DK], BF16, name="vsb")
            st["ctx"] = pers.tile([128, 2, T], BF16, name="ctx")
            st["acc"] = pers.tile([128, 8, 512], F32, name="acc")
            st["ones"] = pers.tile([1, QB], BF16, name="ones")
            if add_bias:
                st["bq_sb"] = pers.tile([1, FPC], BF16, name="bq_sb")
                st["bk_sb"] = pers.tile([1, FPC], BF16, name="bk_sb")
                st["bv_sb"] = pers.tile([1, FPC], BF16, name="bv_sb")
                st["bout_sb"] = pers.tile([1, D], BF16, name="bout_sb")

            nc.vector.memset(st["ones"][:], 1.0)
            nc.vector.memset(st["vsb"][:, :, :, DK:2 * DK], 1.0)

            # weights: once (persist across reps)
            for ko in range(8):
                nc.sync.dma_start(st["wk_sb"][:, ko, :],
                                  wk[ko * 128:(ko + 1) * 128, :])
                nc.sync.dma_start(st["wq_sb"][:, ko, :],
                                  wq[ko * 128:(ko + 1) * 128, :])
            for ko in range(8):
                nc.sync.dma_start(st["wv_sb"][:, ko, :],
                                  wv[ko * 128:(ko + 1) * 128, :])
            if add_bias:
                nc.sync.dma_start(st["bq_sb"][:], bq[:, :])
                nc.sync.dma_start(st["bk_sb"][:], bk[:, :])
                nc.sync.dma_start(st["bv_sb"][:], bv[:, :])
                nc.sync.dma_start(st["bout_sb"][:], bout[:, :])
            for ko in range(8):
                nc.sync.dma_start(st["wout_sb"][:, ko, :],
                                  wout[ko * 128:(ko + 1) * 128, :])

            if debug:
                dbg = {
                    "dbg_a2a": nc.dram_tensor("dbg_a2a", [2, NCORES * 128, TOUT],
                                              BF16, kind="ExternalOutput"),
                    "dbg_qT": nc.dram_tensor("dbg_qT", [128, 2, T], BF16,
                                             kind="ExternalOutput"),
                    "dbg_kT": nc.dram_tensor("dbg_kT", [128, 2, T], BF16,
                                             kind="ExternalOutput"),
                    "dbg_vsb": nc.dram_tensor("dbg_vsb",
                                              [128, NKC, HPC, 2 * DK], BF16,
                                              kind="ExternalOutput"),
                    "dbg_ctx": nc.dram_tensor("dbg_ctx", [128, 2, T], BF16,
                                              kind="ExternalOutput"),
                }
                st["dbg"] = dbg
            for _rep in range(reps):
                _emit_body(nc, tc, dram, st, xT, out, add_bias)
            if debug:
                nc.sync.dma_start(st["dbg"]["dbg_qT"][:, :, :], st["qT"][:])
                nc.sync.dma_start(st["dbg"]["dbg_kT"][:, :, :], st["kT"][:])
                nc.sync.dma_start(st["dbg"]["dbg_vsb"][:, :, :, :], st["vsb"][:])
                nc.sync.dma_start(st["dbg"]["dbg_ctx"][:, :, :], st["ctx"][:])

    nc.finalize()
    return nc


def _emit_body(nc, tc, dram, st, xT, out, add_bias):
    xT_sb, qT, kT, vsb, ctx, acc = (st["xT_sb"], st["qT"], st["kT"],
                                    st["vsb"], st["ctx"], st["acc"])
    wq_sb, wk_sb, wv_sb, wout_sb = (st["wq_sb"], st["wk_sb"], st["wv_sb"],
                                    st["wout_sb"])
    ones = st["ones"]
    Exp = mybir.ActivationFunctionType.Exp
    MULT = mybir.AluOpType.mult
    ADD = mybir.AluOpType.add

    a2a_in = [dram.tile([NCORES * 128, TOUT], BF16, name=f"a2a_in{p}")
              for p in range(2)]
    a2a_out = [dram.tile([NCORES * 128, TOUT], BF16, name=f"a2a_out{p}")
               for p in range(2)]

    with (
        tc.tile_pool(name="rotp", bufs=2, space="PSUM") as rotp,
        tc.tile_pool(name="pvp", bufs=1, space="PSUM") as pvp,
        tc.tile_pool(name="prp", bufs=3) as prp,
        tc.tile_pool(name="ep", bufs=2) as ep,
        tc.tile_pool(name="stp", bufs=2) as stp,
        tc.tile_pool(name="rcpp", bufs=2) as rcpp,
        tc.tile_pool(name="bcp", bufs=2) as bcp,
        tc.tile_pool(name="ctinp", bufs=16) as ctinp,
        tc.tile_pool(name="ostgp", bufs=3) as ostgp,
    ):
        # ---------------- input DMA (qb-major for early compute start)
        for qb in range(NQB):
            for ko in range(8):
                nc.sync.dma_start(
                    xT_sb[:, ko, qb * QB:(qb + 1) * QB],
                    xT[ko * 128:(ko + 1) * 128, qb * QB:(qb + 1) * QB])

        exp_ops = _EXP2_OPS if "D" in EXP_PATTERN else None

        def rot_tile():
            return rotp.tile([128, G, QB], F32, name="rot", tag="rot")

        def proj_mms(sub, wmat, p, qb, bias_t):
            """q/k proj unit: sub [128, 512] <- w_p.T @ xT (8 ko chunks)."""
            if add_bias:
                nc.tensor.matmul(sub, bias_t[:, p * 128:(p + 1) * 128],
                                 ones[:, :QB], start=True, stop=False)
            for ko in range(8):
                nc.tensor.matmul(
                    sub,
                    wmat[:, ko, p * 128:(p + 1) * 128],
                    xT_sb[:, ko, qb * QB:(qb + 1) * QB],
                    start=(ko == 0 and not add_bias), stop=(ko == 7))

        def unit_qk(mat, p, qb):
            dst = qT if mat == "q" else kT
            wmat = wq_sb if mat == "q" else wk_sb
            bias_t = st.get("bq_sb" if mat == "q" else "bk_sb")
            rt = rot_tile()
            sub = rt[:, 0, :]
            proj_mms(sub, wmat, p, qb, bias_t)
            nc.vector.tensor_copy(dst[:, p, qb * QB:(qb + 1) * QB], sub)

        def unit_qk2(mat, p, qb2):
            """q/k proj for qb pair (2*qb2, 2*qb2+1), ko-outer so the
            stationary w chunk is reused across the two 512-col MMs."""
            dst = qT if mat == "q" else kT
            wmat = wq_sb if mat == "q" else wk_sb
            bias_t = st.get("bq_sb" if mat == "q" else "bk_sb")
            rt = rot_tile()
            if add_bias:
                for j in range(2):
                    nc.tensor.matmul(rt[:, j, :],
                                     bias_t[:, p * 128:(p + 1) * 128],
                                     ones[:, :QB], start=True, stop=False)
            for ko in range(8):
                for j in range(2):
                    nc.tensor.matmul(
                        rt[:, j, :],
                        wmat[:, ko, p * 128:(p + 1) * 128],
                        xT_sb[:, ko, (2 * qb2 + j) * QB:(2 * qb2 + j + 1) * QB],
                        start=(ko == 0 and not add_bias), stop=(ko == 7))
            for j in range(2):
                qb = 2 * qb2 + j
                nc.vector.tensor_copy(dst[:, p, qb * QB:(qb + 1) * QB],
                                      rt[:, j, :])

        def unit_v(p, quarter):
            rt = rot_tile()
            for tq in range(4):
                t = quarter * 4 + tq
                sub = rt[:, 0, tq * 128:(tq + 1) * 128]
                if add_bias:
                    nc.tensor.matmul(sub, ones[:, :128],
                                     st["bv_sb"][:, p * 128:(p + 1) * 128],
                                     start=True, stop=False)
                for ko in range(8):
                    nc.tensor.matmul(
                        sub,
                        xT_sb[:, ko, t * 128:(t + 1) * 128],
                        wv_sb[:, ko, p * 128:(p + 1) * 128],
                        start=(ko == 0 and not add_bias), stop=(ko == 7))
            nc.vector.tensor_copy(
                vsb[:, quarter * 4:quarter * 4 + 4, 2 * p:2 * p + 2, 0:DK],
                rt[:, 0, :].rearrange("p (t h d) -> p t h d", t=4, h=2, d=DK))

        ctin = {}

        def unit_op(phase, b, t2, nf):
            """out-proj unit: pair-0 partial (-> acc) or final (+acc -> out)."""
            rt = rot_tile()
            sub = rt[:, 0, :]
            p = 0 if phase == "partial" else 1
            fos = [2 * g_ + p for g_ in range(4)]
            first = True
            if phase == "final" and add_bias:
                nc.tensor.matmul(sub, ones[:, :128],
                                 st["bout_sb"][:, nf * 512:(nf + 1) * 512],
                                 start=True, stop=False)
                first = False
            for i, fo in enumerate(fos):
                nc.tensor.matmul(
                    sub,
                    ctin[(b, fo)][:, t2 * 128:(t2 + 1) * 128],
                    wout_sb[:, fo, nf * 512:(nf + 1) * 512],
                    start=(i == 0 and first), stop=(i == 3))
            idx = b * 4 + t2 * 2 + nf
            if phase == "partial":
                nc.vector.tensor_copy(acc[:, idx, :], sub)
            else:
                og = ostgp.tile([128, 512], F32, name="ostg", tag="ostg")
                nc.vector.tensor_tensor(og[:], sub, acc[:, idx, :], ADD)
                nc.sync.dma_start(
                    out[b, t2 * 128:(t2 + 1) * 128, nf * 512:(nf + 1) * 512],
                    og[:])

        # ---------------- filler queue (writers must precede their readers
        # in emission order; overlap comes from the engine-level schedule)
        filler = deque()
        for qb in range(1, NQB):
            filler.append(("qk", "q", 0, qb))
        for qb in range(NQB):
            filler.append(("qk", "k", 1, qb))
        for qb in range(NQB):
            filler.append(("qk", "q", 1, qb))
        for quarter in range(4):
            filler.append(("v", 1, quarter))

        def run_unit(u):
            kind = u[0]
            if kind == "qk":
                unit_qk(u[1], u[2], u[3])
            elif kind == "qk2":
                unit_qk2(u[1], u[2], u[3])
            elif kind == "v":
                unit_v(u[1], u[2])
            elif kind == "op":
                unit_op(u[1], u[2], u[3], u[4])
            elif kind == "ctin":
                _, p = u
                for b in range(B):
                    for g_ in range(4):
                        fo = 2 * g_ + p
                        t_ = ctinp.tile([128, TOUT], BF16,
                                        name=f"ctin{b}_{fo}", tag="ctin")
                        nc.sync.dma_start(
                            t_[:],
                            a2a_out[p][(b * 4 + g_) * 128:(b * 4 + g_ + 1) * 128, :])
                        ctin[(b, fo)] = t_

        # ---------------- prologue: K(p0) all, Q(p0, qb0)
        for qb in range(NQB):
            unit_qk("k", 0, qb)
        unit_qk("q", 0, 0)

        # ---------------- phase 2
        expctr = [0]

        def emit_exp(n, rt, pr):
            eng = EXP_PATTERN[expctr[0] % len(EXP_PATTERN)]
            expctr[0] += 1
            if eng == "A" or exp_ops is None:
                nc.scalar.activation(pr[:, 0:n, :], rt[:, 0:n, :], Exp,
                                     scale=LN2)
            else:
                et = ep.tile([128, G * QB], F32, name="e2", tag="e2")
                nc.vector._custom_dve(
                    exp_ops["A"], out=et[:, 0:n * QB], in0=rt[:, 0:n, :],
                    s0=MAGIC, s1=E2A_C1, imm2=E2A_C2)
                ebf = et[:].bitcast(BF16)
                nc.vector._custom_dve(
                    exp_ops["B"], out=pr[:, 0:n, :], in0=rt[:, 0:n, :],
                    in1=ebf[:, 0:2 * n * QB:2],
                    s0=MAGIC, s1=E2B_B, imm2=E2B_A)

        def emit_pv(p, grp, pr, pvt):
            for j, (kc, h) in enumerate(grp):
                if SPLIT_PV:
                    for half in range(2):
                        nc.tensor.matmul(
                            pvt[h][:, half, :],
                            vsb[64 * half:64 * half + 64, kc, 2 * p + h, :],
                            pr[64 * half:64 * half + 64, j, :],
                            start=(kc == 0), stop=(kc == NKC - 1))
                else:
                    nc.tensor.matmul(
                        pvt[h][:, 0, :],
                        vsb[:, kc, 2 * p + h, :],
                        pr[:, j, :],
                        start=(kc == 0), stop=(kc == NKC - 1))

        nsp = 2 if SPLIT_PV else 1
        fill_tick = [0]

        def maybe_fill():
            fill_tick[0] += 1
            if fill_tick[0] % FILLER_CADENCE == 0 and filler:
                run_unit(filler.popleft())

        for p in range(2):
            if p == 1:
                # all pair-1 proj writers must precede pair-1 readers
                while filler and filler[0][0] in ("qk", "qk2", "v"):
                    run_unit(filler.popleft())
            for qb in range(NQB):
                slices = [(kc, h) for kc in range(NKC) for h in range(2)]
                groups = [slices[i:i + G] for i in range(0, len(slices), G)]
                hist = deque()  # (grp, rot, pr)
                pvt = {h: pvp.tile([128, nsp, QB], F32,
                                   name=f"pv{h}", tag=f"pv{h}")
                       for h in range(2)}
                for gi, grp in enumerate(groups):
                    if p == 0 and qb == 0 and gi in (0, 2, 4, 6):
                        # vsb quarter q written before the PV that reads it
                        unit_v(0, gi // 2)
                    rt = rot_tile()
                    for j, (kc, h) in enumerate(grp):
                        nc.tensor.matmul(
                            rt[:, j, :],
                            kT[64 * h:64 * h + 64, p, kc * 128:(kc + 1) * 128],
                            qT[64 * h:64 * h + 64, p, qb * QB:(qb + 1) * QB],
                            start=True, stop=True)
                    pr = prp.tile([128, G, QB], BF16, name="pr", tag="pr")
                    hist.append((grp, rt, pr))
                    if len(hist) >= 2:
                        g2, r2, p2 = hist[-2]
                        emit_exp(len(g2), r2, p2)
                    if len(hist) >= 3:
                        g3, _, p3 = hist.popleft()
                        emit_pv(p, g3, p3, pvt)
                    maybe_fill()
                # drain
                g2, r2, p2 = hist[-1]
                emit_exp(len(g2), r2, p2)
                while hist:
                    g3, _, p3 = hist.popleft()
                    emit_pv(p, g3, p3, pvt)
                # normalize
                for h in range(2):
                    if SPLIT_PV:
                        s_ = stp.tile([128, QB], F32, name="st", tag="st")
                        nc.vector.tensor_tensor(s_[:], pvt[h][:, 0, :],
                                                pvt[h][:, 1, :], ADD)
                        base = s_[:]
                    else:
                        base = pvt[h][:, 0, :]
                    # rows 64-127 hold the denominator (64 ones-columns in V)
                    den = rcpp.tile([DK, QB], F32, name="den", tag="den")
                    nc.vector.tensor_copy(den[:], base[DK:2 * DK, :])
                    rcp_t = bcp.tile([DK, QB], F32, name="rcp2", tag="rcp2")
                    nc.vector.reciprocal_approx_fast(rcp_t[:], den[:])
                    nc.vector.tensor_tensor(
                        ctx[64 * h:64 * h + 64, p, qb * QB:(qb + 1) * QB],
                        base[0:DK, :], rcp_t[:], MULT)
            # pair done: exchange token-wise
            for j in range(NCORES):
                nc.sync.dma_start(a2a_in[p][j * 128:(j + 1) * 128, :],
                                  ctx[:, p, j * TOUT:(j + 1) * TOUT])
            nc.gpsimd.collective_compute(
                "AllToAll", mybir.AluOpType.bypass,
                replica_groups=[list(range(NCORES))],
                ins=[a2a_in[p][:].opt()],
                outs=[a2a_out[p][:].opt()])
            if p == 0:
                filler.append(("ctin", 0))
                for b in range(B):
                    for t2 in range(2):
                        for nf in range(2):
                            filler.append(("op", "partial", b, t2, nf))

        # ---------------- tail: drain filler, then final out-proj
        while filler:
            run_unit(filler.popleft())
        run_unit(("ctin", 1))
        if "dbg" in st:
            for p_ in range(2):
                nc.sync.dma_start(st["dbg"]["dbg_a2a"][p_, :, :],
                                  a2a_out[p_][:, :])
        for b in range(B):
            for t2 in range(2):
                for nf in range(2):
                    unit_op("final", b, t2, nf)


# ------------------------------------------------------------------ host
def make_in_maps(x, Wqkv, bqkv, Wout, bout):
    import ml_dtypes
    bf16 = ml_dtypes.bfloat16
    x = np.asarray(x, dtype=np.float32)
    Wqkv = np.asarray(Wqkv, dtype=np.float32)
    bqkv = np.asarray(bqkv, dtype=np.float32)
    Wout = np.asarray(Wout, dtype=np.float32)
    bout = np.asarray(bout, dtype=np.float32)
    add_bias = bool(np.any(bqkv) or np.any(bout))

    qscale = LOG2E / math.sqrt(DK)
    xT_all = np.ascontiguousarray(np.transpose(x, (0, 2, 1))).astype(bf16)
    wout_b = np.ascontiguousarray(Wout).astype(bf16)
    in_maps = []
    for c in range(NCORES):
        b = c // GROUPS
        h0 = HPC * (c % GROUPS)
        fsl = slice(h0 * DK, h0 * DK + FPC)
        m = {
            "xT": xT_all[b],
            "wq": np.ascontiguousarray(Wqkv[:, 0 * D:1 * D][:, fsl]
                                       * qscale).astype(bf16),
            "wk": np.ascontiguousarray(Wqkv[:, 1 * D:2 * D][:, fsl]).astype(bf16),
            "wv": np.ascontiguousarray(Wqkv[:, 2 * D:3 * D][:, fsl]).astype(bf16),
            "wout": wout_b,
        }
        if add_bias:
            m["bq"] = (np.ascontiguousarray(bqkv[0 * D:1 * D][fsl])[None, :]
                       * qscale).astype(bf16)
            m["bk"] = np.ascontiguousarray(
                bqkv[1 * D:2 * D][fsl])[None, :].astype(bf16)
            m["bv"] = np.ascontiguousarray(
                bqkv[2 * D:3 * D][fsl])[None, :].astype(bf16)
            m["bout"] = bout[None, :].astype(bf16)
        in_maps.append(m)
    return in_maps


_CACHE = {}


def _get_runner(reps: int = 1, add_bias: bool = False):
    key = ("runner", reps, add_bias)
    if key in _CACHE:
        return _CACHE[key]

    import jax
    from jax.experimental.shard_map import shard_map
    from jax.sharding import Mesh, PartitionSpec
    from concourse import bass2jax
    from concourse import mybir as _mybir

    nc = build_nc(reps=reps, add_bias=add_bias)
    bass2jax.install_neuronx_cc_hook()

    partition_name = nc.partition_id_tensor.name if nc.partition_id_tensor else None
    in_names, out_names, out_avals = [], [], []
    for alloc in nc.m.functions[0].allocations:
        if not isinstance(alloc, _mybir.MemoryLocationSet):
            continue
        name = alloc.memorylocations[0].name
        if alloc.kind == "ExternalInput":
            if name != partition_name:
                in_names.append(name)
        elif alloc.kind == "ExternalOutput":
            out_names.append(name)
            out_avals.append(
                jax.core.ShapedArray(
                    tuple(alloc.tensor_shape), _mybir.dt.np(alloc.dtype)))
    n_params = len(in_names)
    all_in_names = list(in_names) + list(out_names)
    if partition_name is not None:
        all_in_names.append(partition_name)

    def _body(*args):
        operands = list(args)
        if partition_name is not None:
            operands.append(bass2jax.partition_id_tensor())
        outs = bass2jax._bass_exec_p.bind(
            *operands,
            out_avals=tuple(out_avals),
            in_names=tuple(all_in_names),
            out_names=tuple(out_names),
            lowering_input_output_aliases=(),
            sim_require_finite=False,
            sim_require_nnan=False,
            nc=nc,
        )
        return tuple(outs)

    devices = jax.devices()[:NCORES]
    mesh = Mesh(np.asarray(devices), ("core",))
    n_outs = len(out_names)
    fn = jax.jit(
        shard_map(
            _body,
            mesh=mesh,
            in_specs=(PartitionSpec("core"),) * (n_params + n_outs),
            out_specs=(PartitionSpec("core"),) * n_outs,
            check_rep=False,
        ),
        keep_unused=True,
    )

    def run(in_maps):
        concat_in = [
            np.concatenate([np.asarray(in_maps[c][nm]) for c in range(NCORES)],
                           axis=0)
            for nm in in_names
        ]
        zeros = [
            np.zeros((NCORES * av.shape[0], *av.shape[1:]), av.dtype)
            for av in out_avals
        ]
        out_arrs = fn(*concat_in, *zeros)
        return [
            {nm: np.asarray(out_arrs[i]).reshape(NCORES, *out_avals[i].shape)[c]
             for i, nm in enumerate(out_names)}
            for c in range(NCORES)
        ]

    runner = {"run": run, "fn": fn, "in_names": in_names,
              "out_avals": out_avals, "out_names": out_names,
              "n_params": n_params, "mesh": mesh}
    _CACHE[key] = runner
    return runner


def kernel(x, Wqkv, bqkv, Wout, bout) -> np.ndarray:
    add_bias = bool(np.any(np.asarray(bqkv)) or np.any(np.asarray(bout)))
    runner = _get_runner(add_bias=add_bias)
    in_maps = make_in_maps(x, Wqkv, bqkv, Wout, bout)
    results = runner["run"](in_maps)
    full = np.empty((B, T, D), dtype=np.float32)
    for c in range(NCORES):
        full[:, c * TOUT:(c + 1) * TOUT, :] = results[c]["out"]
    return full
